# revision 1
# baseline (speedup 1.0000x reference)
"""Trainium2 Bass kernel for EncoderWithPositionalAttentionLayer.

Sharding: data-parallel over batch B=8 across 8 NeuronCores (one batch
element per core).  The batch-independent relative-position tensors are
algebraically collapsed:

  kr[i,j,:]    = rel_enc[idx(i,j)] @ wkr + bkr,  idx = clip(j-i,-L,L)+L
  biasprod     = q_h . (rel_enc @ wkr_h)[idx]  (+ softmax-invariant consts)
  bias1        = (rel_enc @ wkr @ wb1)[idx,h]  (+ softmax-invariant consts)

Under the causal mask idx in [0,100].  Terms constant along a score row
(q.bkr_h, bb0, bb1, clip-value tables at idx=0) cancel in softmax and are
dropped.  What remains is a banded score bias
    Band[i,j] = Db[i, j-i+100]   for j in [i-99, i]
    Db[i,t]   = (q_h[i]/8) . (8*RWD_h[:,t]) + E1D[t,h]
with RWD/E1D tiny delta tables (vs idx=0) computed once on-device.  Db is
scattered into score tiles through a per-(head,itile) DRAM region using a
row-stride-228 skew write over a 227-wide window; the window's fill
pattern also carries the causal mask (-60 above the diagonal), so one
window read + one vector add applies relative bias AND causal mask.

Score layout [i x j], softmax over the free dim (denominator via the Exp
activation's accum_out), attention probabilities transposed per 128-block
on the PE for the attn @ v contraction.  All matmuls run as float32r
(full-rate on TRN2 for moving dim >= 256).
"""

import contextlib
import sys

sys.path.insert(0, "/opt/trn_rl_repo")

import numpy as np

import concourse.bass as bass
from concourse import bacc
import concourse.mybir as mybir
import concourse.tile as tile

F32 = mybir.dt.float32
F32R = mybir.dt.float32r
AF = mybir.ActivationFunctionType
ALU = mybir.AluOpType

B, T, D, H, HID = 8, 512, 512, 8, 2048
DIM = D // H          # 64
L = 100
NT = L + 1            # idx values used under causal mask: 0..100
BW = L                # band width (t = 1..100)
WIN = 227             # window width per 128-row i-tile: 128 + 99
EPS = 1e-3
P = 128
TS = T // P           # 4
DS = D // P           # 4
CS = HID // P         # 16
NEG = -60.0           # exp(-60) ~ 1e-26: exact-enough masking, safe on HW


def r32(ap):
    return ap.bitcast(F32R)


def rf(ap):
    return ap.bitcast(F32)


def build_nc():
    nc = bacc.Bacc()

    dp = nc.declare_dram_parameter
    values = dp("values_b", [T, D], F32, isOutput=False)
    maskb = dp("maskbias_b", [T], F32, isOutput=False)       # 0 / NEG per j
    rel101 = dp("rel101", [NT, D], F32, isOutput=False)      # rel_enc[:101]
    ln0_g = dp("ln0_g", [D], F32, isOutput=False)
    ln0_b = dp("ln0_b", [D], F32, isOutput=False)
    w_h0 = dp("w_h0", [D, HID], F32, isOutput=False)
    b_h0 = dp("b_h0", [HID], F32, isOutput=False)
    wq = dp("wq", [HID, D], F32, isOutput=False)
    bq = dp("bq", [D], F32, isOutput=False)
    wke = dp("wke", [HID, D], F32, isOutput=False)
    bke = dp("bke", [D], F32, isOutput=False)
    wkv = dp("wkv", [HID, D], F32, isOutput=False)
    bkv = dp("bkv", [D], F32, isOutput=False)
    wkr = dp("wkr", [D, D], F32, isOutput=False)
    wb0 = dp("wb0", [D, H], F32, isOutput=False)
    wb1 = dp("wb1", [D, H], F32, isOutput=False)
    ln1_g = dp("ln1_g", [D], F32, isOutput=False)
    ln1_b = dp("ln1_b", [D], F32, isOutput=False)
    w_h1 = dp("w_h1", [D, HID], F32, isOutput=False)
    b_h1 = dp("b_h1", [HID], F32, isOutput=False)
    w_o1 = dp("w_o1", [HID, D], F32, isOutput=False)
    b_o1 = dp("b_o1", [D], F32, isOutput=False)
    out = dp("out_b", [T, D], F32, isOutput=True)

    def pcol(v):     # [D] -> [128, DS] partition-major
        return v.rearrange("(s p) -> p s", p=P)

    with tile.TileContext(nc) as tc, contextlib.ExitStack() as ctx:
        consts = ctx.enter_context(tc.tile_pool(name="consts", bufs=1))
        persist = ctx.enter_context(tc.tile_pool(name="persist", bufs=1))
        wpool = ctx.enter_context(tc.tile_pool(name="wpool", bufs=3))
        work = ctx.enter_context(tc.tile_pool(name="work", bufs=3))
        psum = ctx.enter_context(tc.tile_pool(name="psum", bufs=3, space="PSUM"))
        psacc = ctx.enter_context(tc.tile_pool(name="psacc", bufs=4, space="PSUM"))
        dram = ctx.enter_context(tc.tile_pool(name="dram", bufs=1, space="DRAM"))

        # Observer: a 1x1 matmul reading `ap` absorbs that producer's
        # semaphore into the PE's vector clock.  fp32(r) matmuls are
        # self-loading single instructions with a single sync-wait slot, so
        # every matmul may carry at most ONE not-yet-observed semaphore.
        junk_ps = psum.tile([1, 8], F32, name="junk_ps", tag="junk", bufs=1)

        def observe(ap):
            sl = ap
            while len(sl.shape) > 2:
                sl = sl[:, 0]
            nc.tensor.matmul(junk_ps[0:1, 0:1], rf(sl[0:1, 0:1]),
                             rf(sl[0:1, 0:1]), start=True, stop=True)

        # ----------------------------- constants -----------------------------
        ident = consts.tile([P, P], F32)
        from concourse.masks import make_identity
        make_identity(nc, ident)
        eps_sb = consts.tile([P, 1], F32)
        nc.vector.memset(eps_sb, EPS)
        ones_f = consts.tile([1, P], F32)
        nc.vector.memset(ones_f, 1.0)
        ones_col = consts.tile([1, P], F32R)
        nc.scalar.copy(ones_col, ones_f)
        eighth_col = consts.tile([1, P], F32)
        nc.vector.memset(eighth_col, 0.125)
        # Window fill: F[p, w] = NEG where w > p+99 (j > i), else 0
        fillF = consts.tile([P, WIN], F32)
        nc.vector.memset(fillF, 0.0)
        nc.gpsimd.affine_select(
            out=fillF, in_=fillF, compare_op=ALU.is_ge, fill=NEG,
            base=99, channel_multiplier=1, pattern=[[-1, WIN]])

        # --------------------------- small loads -----------------------------
        gT0 = persist.tile([P, DS], F32)
        nc.sync.dma_start(gT0, pcol(ln0_g))
        bT0 = persist.tile([P, DS], F32)
        nc.sync.dma_start(bT0, pcol(ln0_b))
        gT1 = persist.tile([P, DS], F32)
        nc.sync.dma_start(gT1, pcol(ln1_g))
        bT1 = persist.tile([P, DS], F32)
        nc.sync.dma_start(bT1, pcol(ln1_b))
        bh0_t = persist.tile([P, CS], F32)
        nc.sync.dma_start(bh0_t, b_h0.rearrange("(s p) -> p s", p=P))
        bh1_t = persist.tile([P, CS], F32)
        nc.sync.dma_start(bh1_t, b_h1.rearrange("(s p) -> p s", p=P))
        bq_t = persist.tile([P, DS], F32)
        nc.sync.dma_start(bq_t, pcol(bq))
        nc.vector.tensor_scalar_mul(bq_t, bq_t, 0.125)     # q is stored as q/8
        bke_t = persist.tile([P, DS], F32)
        nc.sync.dma_start(bke_t, pcol(bke))
        bkv_bc = persist.tile([P, D], F32)
        nc.gpsimd.dma_start(out=bkv_bc, in_=bass.AP(
            tensor=bkv, offset=0, ap=[[0, P], [1, D]]))
        bo1_bc = persist.tile([P, D], F32)
        nc.gpsimd.dma_start(out=bo1_bc, in_=bass.AP(
            tensor=b_o1, offset=0, ap=[[0, P], [1, D]]))
        mask_row = persist.tile([1, T], F32R)
        nc.sync.dma_start(mask_row, r32(maskb[None, :]))
        wb0_sb = persist.tile([P, DS, H], F32R)
        nc.sync.dma_start(wb0_sb, r32(wb0.rearrange("(s p) h -> p s h", p=P)))
        wb1_sb = persist.tile([P, DS, H], F32)
        nc.sync.dma_start(wb1_sb, wb1.rearrange("(s p) h -> p s h", p=P))
        wkr_sb = persist.tile([P, DS, D], F32, name="wkr_sb", tag="pdmat")
        nc.sync.dma_start(wkr_sb, wkr.rearrange("(s p) d -> p s d", p=P))
        r101 = persist.tile([NT, D], F32)
        nc.sync.dma_start(r101, rel101[:, :])

        # ---------------- relative tables (batch-independent) ----------------
        observe(ident)
        observe(r101)
        # RT8 [128(e), DS, 101] = (8 * rel101).T
        rt8 = persist.tile([P, DS, NT], F32)
        for es in range(DS):
            tp = psum.tile([P, NT], F32, name="tp", tag="pp")
            nc.tensor.matmul(tp, r101[:, es * P:(es + 1) * P], ident[:NT, :NT],
                             start=True, stop=True)
            nc.vector.tensor_scalar_mul(rt8[:, es, :], tp, 8.0)
        # RWT8 [128(d), DS, 101] = (8 R @ wkr).T
        observe(wkr_sb)
        rwt8 = persist.tile([P, DS, NT], F32)
        for dsub in range(DS):
            pp = psum.tile([P, NT], F32, name="pp1", tag="pp")
            for es in range(DS):
                nc.tensor.matmul(pp, wkr_sb[:, es, dsub * P:(dsub + 1) * P],
                                 rt8[:, es, :],
                                 start=(es == 0), stop=(es == DS - 1))
            nc.vector.tensor_copy(rwt8[:, dsub, :], pp)
        # E1_8 [101, H] = (8 R @ wkr) @ wb1
        observe(wb1_sb)
        e1p = psum.tile([NT, H], F32, name="e1p", tag="pp")
        for c in range(DS):
            nc.tensor.matmul(e1p, rwt8[:, c, :], wb1_sb[:, c, :],
                             start=(c == 0), stop=(c == DS - 1))
        e1_sb = persist.tile([NT, H], F32)
        nc.vector.tensor_copy(e1_sb, e1p)
        e1tp = psum.tile([H, NT], F32, name="e1tp", tag="pp")
        nc.tensor.matmul(e1tp, e1_sb, ident[:NT, :NT], start=True, stop=True)
        e1t = persist.tile([H, NT], F32)
        nc.vector.tensor_copy(e1t, e1tp)
        # E1DT_8 [H, 100] = E1T8[:,1:] - E1T8[:,0]
        e1dt = persist.tile([H, BW], F32)
        nc.vector.tensor_tensor(e1dt, e1t[:, 1:NT],
                                e1t[:, 0:1].to_broadcast((H, BW)), ALU.subtract)
        # RWD8 [128(d), DS, 100] = RWT8[:,:,1:] - RWT8[:,:,0]
        rwd8 = persist.tile([P, DS, BW], F32)
        nc.vector.tensor_tensor(rwd8, rwt8[:, :, 1:NT],
                                rwt8[:, :, 0:1].to_broadcast((P, DS, BW)),
                                ALU.subtract)
        # E1D rows relocated to partition 0 (K=1 matmul rhs)
        e1drows = []
        for h in range(H):
            t_ = persist.tile([1, BW], F32, name=f"e1dr{h}")
            nc.gpsimd.dma_start(t_, e1dt[h:h + 1, :])
            e1drows.append(t_)

        # --------------------------- LN helper --------------------------------
        def layernorm_to_T(x_tiles, gT, bT, lnT_out, name):
            for tt in range(TS):
                xt = x_tiles[:, tt, :]
                stats = work.tile([P, 6], F32, name=f"{name}st{tt}", tag="lnst")
                nc.vector.bn_stats(out=stats, in_=xt)
                mv = work.tile([P, 2], F32, name=f"{name}mv{tt}", tag="lnmv")
                nc.vector.bn_aggr(out=mv, in_=stats)
                rstd = work.tile([P, 1], F32, name=f"{name}rs{tt}", tag="lnrs")
                nc.scalar.activation(out=rstd, in_=mv[:, 1:2], func=AF.Sqrt,
                                     bias=eps_sb, scale=1.0)
                nc.vector.reciprocal(rstd, rstd)
                xn = work.tile([P, T], F32, name=f"{name}xn{tt}", tag="lnxn")
                nc.vector.tensor_scalar(xn, xt, mv[:, 0:1], rstd,
                                        op0=ALU.subtract, op1=ALU.mult)
                for es in range(DS):
                    tp = psum.tile([P, P], F32, name=f"{name}tp", tag="pp")
                    nc.tensor.matmul(tp, xn[:, es * P:(es + 1) * P], ident,
                                     start=True, stop=True)
                    nc.vector.tensor_scalar(
                        lnT_out[:, es, tt * P:(tt + 1) * P], tp,
                        gT[:, es:es + 1], bT[:, es:es + 1],
                        op0=ALU.mult, op1=ALU.add)

        # ------------------------- values + LN0 ------------------------------
        vals = persist.tile([P, TS, D], F32)
        nc.sync.dma_start(vals, values.rearrange("(s p) d -> p s d", p=P))
        ln0T = persist.tile([P, DS, T], F32R, name="ln0T", tag="lnT")
        layernorm_to_T(vals, gT0, bT0, ln0T, "ln0")

        # ------------------ block0: xT = relu(w_h0.T @ ln0T) -----------------
        wh0r = w_h0.rearrange("(s p) c -> p s c", p=P)
        xT = persist.tile([P, CS, T], F32R, name="xT", tag="xT")
        for cg in range(4):
            wch = wpool.tile([P, DS, 512], F32R, name="wch", tag="wchunk")
            nc.sync.dma_start(wch, r32(wh0r[:, :, cg * 512:(cg + 1) * 512]))
            observe(wch)
            for lc in range(4):
                cs_ = cg * 4 + lc
                pp = psum.tile([P, T], F32, name="h0pp", tag="pp")
                for es in range(DS):
                    nc.tensor.matmul(pp, wch[:, es, lc * P:(lc + 1) * P],
                                     ln0T[:, es, :],
                                     start=(es == 0), stop=(es == DS - 1))
                nc.scalar.activation(out=xT[:, cs_, :], in_=pp, func=AF.Relu,
                                     bias=bh0_t[:, cs_:cs_ + 1], scale=1.0)

        # --------------------------- projections -----------------------------
        def project_T(w_dram, dest, bias_col, scale):
            """dest [128(d), DS, T] = (x @ w + b).T (scaled); w [HID, D]."""
            wr = w_dram.rearrange("(s p) d -> p s d", p=P)
            accs = [psacc.tile([P, T], F32, name=f"pa{d}", tag="acc")
                    for d in range(DS)]
            for cg in range(4):
                wch = wpool.tile([P, 4, D], F32R, name="wpch", tag="wchunk")
                nc.sync.dma_start(wch, r32(wr[:, cg * 4:(cg + 1) * 4, :]))
                observe(wch)
                for lc in range(4):
                    cs_ = cg * 4 + lc
                    for dsub in range(DS):
                        nc.tensor.matmul(
                            accs[dsub], wch[:, lc, dsub * P:(dsub + 1) * P],
                            xT[:, cs_, :],
                            start=(cs_ == 0), stop=(cs_ == CS - 1))
            for dsub in range(DS):
                nc.scalar.activation(out=dest[:, dsub, :], in_=accs[dsub],
                                     func=AF.Identity,
                                     bias=bias_col[:, dsub:dsub + 1], scale=scale)

        qT = persist.tile([P, DS, T], F32R)          # holds q/8 transposed
        project_T(wq, qT, bq_t, 0.125)
        keT = persist.tile([P, DS, T], F32R)
        project_T(wke, keT, bke_t, 1.0)

        # kv natural layout [128(t), TS, D]
        kv = persist.tile([P, TS, D], F32R, name="kv", tag="pdmat")
        wkvr = wkv.rearrange("(s p) d -> p s d", p=P)
        kvaccs = [psacc.tile([P, D], F32, name=f"kva{t}", tag="acc")
                  for t in range(TS)]
        for cg in range(4):
            wch = wpool.tile([P, 4, D], F32R, name="wkch", tag="wchunk")
            nc.sync.dma_start(wch, r32(wkvr[:, cg * 4:(cg + 1) * 4, :]))
            observe(wch)
            for lc in range(4):
                cs_ = cg * 4 + lc
                for tt in range(TS):
                    nc.tensor.matmul(kvaccs[tt],
                                     xT[:, cs_, tt * P:(tt + 1) * P],
                                     wch[:, lc, :],
                                     start=(cs_ == 0), stop=(cs_ == CS - 1))
        for tt in range(TS):
            nc.scalar.copy(kv[:, tt, :], kvaccs[tt])   # bkv added into v1 below

        # ---------------- bias0 rows (+ mask): [H, T] then per-head [1, T] ----
        observe(wb0_sb)
        observe(mask_row)
        b0p = psum.tile([H, T], F32, name="b0p", tag="pp")
        for c in range(DS):
            nc.tensor.matmul(b0p, wb0_sb[:, c, :], keT[:, c, :],
                             start=(c == 0), stop=False)
        nc.tensor.matmul(b0p, ones_col[:, 0:H], mask_row,
                         start=False, stop=True)
        b0m_sb = persist.tile([H, T], F32R)
        nc.scalar.copy(b0m_sb, b0p)

        # ------------------------------ attention -----------------------------
        attn_out = persist.tile([P, TS, D], F32)
        hd = lambda h: (h % 2) * DIM          # partition offset of head h
        for h in range(H):
            b0row = work.tile([1, T], F32R, name=f"b0r{h}", tag="b0row")
            nc.gpsimd.dma_start(b0row, b0m_sb[h:h + 1, :])
            observe(b0row)
            observe(e1drows[h])
            scr = []
            for ti in range(TS):
                sc = dram.tile([P * WIN], F32, name=f"scr_h{h}_t{ti}")
                nc.sync.dma_start(
                    bass.AP(tensor=sc.tensor, offset=sc.offset,
                            ap=[[WIN, P], [1, WIN]]),
                    fillF)
                scr.append(sc)
            for ti in range(TS):
                # Db[i, t] = (q/8) . 8RWD (+ E1D via K=1)
                dbp = psum.tile([P, BW], F32, name="dbp", tag="pp")
                nc.tensor.matmul(
                    dbp, rf(qT[hd(h):hd(h) + DIM, h // 2, ti * P:(ti + 1) * P]),
                    rwd8[hd(h):hd(h) + DIM, h // 2, :],
                    start=True, stop=False)
                nc.tensor.matmul(dbp, eighth_col, e1drows[h],
                                 start=False, stop=True)
                db_sb = work.tile([P, BW], F32, name="db_sb", tag="db_sb")
                nc.scalar.copy(db_sb, dbp)
                # skew write: offset = p*228 + (t-1)  =>  w = p + (t-1)
                nc.sync.dma_start(
                    bass.AP(tensor=scr[ti].tensor, offset=scr[ti].offset,
                            ap=[[WIN + 1, P], [1, BW]]),
                    db_sb)
            for ti in range(TS):
                nj = (ti + 1) * P
                sp = psacc.tile([P, T], F32, name="sp", tag="acc")
                nc.tensor.matmul(
                    sp[:, 0:nj],
                    qT[hd(h):hd(h) + DIM, h // 2, ti * P:(ti + 1) * P],
                    keT[hd(h):hd(h) + DIM, h // 2, 0:nj],
                    start=True, stop=False)
                nc.tensor.matmul(sp[:, 0:nj], ones_col,
                                 b0row[:, 0:nj], start=False, stop=True)
                wread = work.tile([P, WIN], F32, name="wread", tag="wread")
                nc.sync.dma_start(
                    wread,
                    bass.AP(tensor=scr[ti].tensor, offset=scr[ti].offset,
                            ap=[[WIN, P], [1, WIN]]))
                j0 = ti * P - 99
                if j0 < 0:
                    nc.vector.tensor_add(sp[:, 0:P], sp[:, 0:P], wread[:, 99:WIN])
                else:
                    nc.vector.tensor_add(sp[:, j0:j0 + WIN], sp[:, j0:j0 + WIN],
                                         wread)
                at = work.tile([P, T], F32, name="at", tag="at")
                zs = work.tile([P, 1], F32, name="zs", tag="zs")
                nc.scalar.activation(out=at[:, 0:nj], in_=sp[:, 0:nj],
                                     func=AF.Exp, bias=0.0, scale=1.0,
                                     accum_out=zs)
                rz = work.tile([P, 1], F32, name="rz", tag="rz")
                nc.vector.reciprocal(rz, zs)
                op = psacc.tile([P, DIM], F32, name="avp", tag="acc")
                for js in range(ti + 1):
                    tp = psum.tile([P, P], F32, name="attp", tag="pp")
                    nc.tensor.matmul(tp, at[:, js * P:(js + 1) * P], ident,
                                     start=True, stop=True)
                    atT = work.tile([P, P], F32R, name="atT", tag="atT")
                    nc.scalar.copy(atT, tp)
                    nc.tensor.matmul(op, atT,
                                     kv[:, js, h * DIM:(h + 1) * DIM],
                                     start=(js == 0), stop=(js == ti))
                nc.scalar.activation(
                    out=attn_out[:, ti, h * DIM:(h + 1) * DIM], in_=op,
                    func=AF.Identity, bias=0.0, scale=rz)

        # ------------------------ residual + block1 ---------------------------
        v1 = persist.tile([P, TS, D], F32)
        for tt in range(TS):
            nc.vector.tensor_add(v1[:, tt, :], vals[:, tt, :], attn_out[:, tt, :])
            nc.vector.tensor_add(v1[:, tt, :], v1[:, tt, :], bkv_bc)
        ln1T = persist.tile([P, DS, T], F32R, name="ln1T", tag="lnT")
        layernorm_to_T(v1, gT1, bT1, ln1T, "ln1")

        wh1r = w_h1.rearrange("(s p) c -> p s c", p=P)
        x1T = persist.tile([P, CS, T], F32R, name="x1T", tag="xT")
        for cg in range(4):
            wch = wpool.tile([P, DS, 512], F32R, name="w1ch", tag="wchunk")
            nc.sync.dma_start(wch, r32(wh1r[:, :, cg * 512:(cg + 1) * 512]))
            observe(wch)
            for lc in range(4):
                cs_ = cg * 4 + lc
                pp = psum.tile([P, T], F32, name="h1pp", tag="pp")
                for es in range(DS):
                    nc.tensor.matmul(pp, wch[:, es, lc * P:(lc + 1) * P],
                                     ln1T[:, es, :],
                                     start=(es == 0), stop=(es == DS - 1))
                nc.scalar.activation(out=x1T[:, cs_, :], in_=pp, func=AF.Relu,
                                     bias=bh1_t[:, cs_:cs_ + 1], scale=1.0)

        wo1r = w_o1.rearrange("(s p) d -> p s d", p=P)
        o1accs = [psacc.tile([P, D], F32, name=f"o1a{t}", tag="acc")
                  for t in range(TS)]
        for cg in range(4):
            wch = wpool.tile([P, 4, D], F32R, name="woch", tag="wchunk")
            nc.sync.dma_start(wch, r32(wo1r[:, cg * 4:(cg + 1) * 4, :]))
            observe(wch)
            for lc in range(4):
                cs_ = cg * 4 + lc
                for tt in range(TS):
                    nc.tensor.matmul(o1accs[tt],
                                     x1T[:, cs_, tt * P:(tt + 1) * P],
                                     wch[:, lc, :],
                                     start=(cs_ == 0), stop=(cs_ == CS - 1))
        outr = out.rearrange("(s p) d -> p s d", p=P)
        for tt in range(TS):
            fin = work.tile([P, D], F32, name="fin", tag="fin")
            nc.vector.tensor_add(fin, o1accs[tt], v1[:, tt, :])
            nc.vector.tensor_add(fin, fin, bo1_bc)
            nc.sync.dma_start(outr[:, tt, :], fin)

    if not nc.is_finalized():
        nc.finalize()
    return nc


_NC_CACHE = None


def kernel(**inputs) -> np.ndarray:
    global _NC_CACHE
    if _NC_CACHE is None:
        _NC_CACHE = build_nc()
    nc = _NC_CACHE

    from concourse.bass_utils import run_bass_kernel_spmd

    mask = np.asarray(inputs["values_mask"])
    maskbias = np.where(mask, 0.0, NEG).astype(np.float32)       # [B, T]
    shared = {"rel101": np.ascontiguousarray(
        np.asarray(inputs["rel_enc"], dtype=np.float32)[:NT])}
    for name in ("ln0_g", "ln0_b", "w_h0", "b_h0", "wq", "bq", "wke", "bke",
                 "wkv", "bkv", "wkr", "wb0", "wb1", "ln1_g", "ln1_b",
                 "w_h1", "b_h1", "w_o1", "b_o1"):
        shared[name] = np.ascontiguousarray(np.asarray(inputs[name],
                                                       dtype=np.float32))
    # bkr folds away exactly (delta band + softmax row-constant cancellation).

    vals = np.asarray(inputs["values"], dtype=np.float32)
    in_maps = []
    for b in range(B):
        m = dict(shared)
        m["values_b"] = np.ascontiguousarray(vals[b])
        m["maskbias_b"] = np.ascontiguousarray(maskbias[b])
        in_maps.append(m)

    res = run_bass_kernel_spmd(nc, in_maps, core_ids=list(range(B)))
    return np.stack([res.results[b]["out_b"] for b in range(B)], axis=0)


if __name__ == "__main__":
    nc = build_nc()
    print("built ok")



# revision 6
# speedup vs baseline: 1.7182x; 1.7182x over previous
"""Trainium2 Bass kernel for EncoderWithPositionalAttentionLayer.

Sharding: data-parallel over batch B=8 across 8 NeuronCores (one batch
element per core).  The batch-independent relative-position algebra is
collapsed on the HOST (exact fp32 numpy):

  score[i,j] = q[i].ke[j]/8 + q[i].RW[:,idx] + E1[idx,h] + b0[j] (+consts)
  idx = clip(j-i,-100,100)+100; under the causal mask idx in [0,100].
  Terms constant along a score row (idx=0 tables, bb0/bb1, bkr terms)
  cancel in softmax.  What remains is a banded bias
     Db[i,t] = (q[i]/8).(8*RWD[:,t]) + E1D[t,h],  t = j-i+100 in [1,100]
  with RWD/E1D host-computed delta tables (vs idx=0).

On device, Db goes through a DRAM scratch with read-side skew: rows of
width 360 per (partition, head, itile) hold [127 zeros][100 Db][133 NEG];
one contiguous write, then a read with partition-dependent offset
(stride ROWS-1) yields the j-aligned causal-masked bias window that a
single DVE add applies to each score tile.

Everything on the main path is bf16 (matmul rate is 1 cycle/row, same
as fp32r, at any moving dim; PSUM accumulation stays fp32).  Weights
are host-prepacked partition-major so every weight DMA is 128
contiguous 16KB runs.
"""

import contextlib
import sys

sys.path.insert(0, "/opt/trn_rl_repo")

import numpy as np
import ml_dtypes

import concourse.bass as bass
from concourse import bacc
import concourse.mybir as mybir
import concourse.tile as tile

F32 = mybir.dt.float32
BF16 = mybir.dt.bfloat16
AF = mybir.ActivationFunctionType
ALU = mybir.AluOpType
NPBF = ml_dtypes.bfloat16

B, T, D, H, HID = 8, 512, 512, 8, 2048
DIM = D // H          # 64
L = 100
BW = L                # band width (t = 1..100)
EPS = 1e-3
P = 128
TS = T // P           # 4
DS = D // P           # 4
CS = HID // P         # 16
NEG = -60.0           # exp(-60) ~ 1e-26: exact-enough masking
WROW = 360            # scratch row: [127 zeros][100 Db][133 NEG]
NK = H * TS           # 32 scratch tiles (k = ti*H + h)
ROWS = NK * WROW      # per-partition scratch row block (11520)
WIN = 227             # j-aligned window width read back per tile

# smalls (fp32) column offsets
SM_LN0G, SM_LN0B, SM_LN1G, SM_LN1B = 0, 4, 8, 12
SM_BH0, SM_BH1 = 16, 32
SM_BQ, SM_BKE = 48, 52
SM_ID32 = 56
SM_BKV = SM_ID32 + 128          # 184
SM_BO1 = SM_BKV + 512           # 696
NS = SM_BO1 + 512               # 1208

# smallsb (bf16) column offsets
SB_ID16 = 0
SB_ONES = 128
SB_RWD = 256                    # [128, 4*100]
SB_E1D = SB_RWD + 400           # row 0: 8 heads x 100
SB_MASK = SB_E1D + 800          # row 0: maskbias [T]
SB_WB0 = SB_MASK + 512          # [128, 4*8]
SB_FILL = SB_WB0 + 32           # [128, 360] scratch row fill pattern
NSB = SB_FILL + WROW            # 2388


def build_nc():
    nc = bacc.Bacc()

    dp = nc.declare_dram_parameter
    values = dp("values_b", [P, TS, D], F32, isOutput=False)
    smalls = dp("smalls", [P, NS], F32, isOutput=False)
    smallsb = dp("smallsb_b", [P, NSB], BF16, isOutput=False)
    wh0p = dp("wh0p", [P, DS, HID], BF16, isOutput=False)
    wqp = dp("wqp", [P, CS, D], BF16, isOutput=False)
    wkep = dp("wkep", [P, CS, D], BF16, isOutput=False)
    wkvp = dp("wkvp", [P, CS, D], BF16, isOutput=False)
    wh1p = dp("wh1p", [P, DS, HID], BF16, isOutput=False)
    wo1p = dp("wo1p", [P, CS, D], BF16, isOutput=False)
    out = dp("out_b", [P, TS, D], F32, isOutput=True)

    with tile.TileContext(nc) as tc, contextlib.ExitStack() as ctx:
        persist = ctx.enter_context(tc.tile_pool(name="persist", bufs=1))
        wpool = ctx.enter_context(tc.tile_pool(name="wpool", bufs=3))
        work = ctx.enter_context(tc.tile_pool(name="work", bufs=3))
        psum = ctx.enter_context(tc.tile_pool(name="psum", bufs=2, space="PSUM"))
        psacc = ctx.enter_context(tc.tile_pool(name="psacc", bufs=4, space="PSUM"))
        pso = ctx.enter_context(tc.tile_pool(name="pso", bufs=2, space="PSUM"))
        dram = ctx.enter_context(tc.tile_pool(name="dram", bufs=1, space="DRAM"))

        # ---------------- input DMAs (scalar ring: small stuff) --------------
        smb = persist.tile([P, NSB], BF16)
        nc.scalar.dma_start(smb, smallsb[:, :])
        sm = persist.tile([P, NS], F32)
        nc.scalar.dma_start(sm, smalls[:, :])
        vals = persist.tile([P, TS, D], F32)
        nc.scalar.dma_start(vals, values[:, :, :])

        # ---------------- weight DMAs (sync ring, use order) -----------------
        wh0 = wpool.tile([P, DS, HID], BF16, name="wh0", tag="w")
        nc.sync.dma_start(wh0, wh0p[:, :, :])
        wq = wpool.tile([P, CS, D], BF16, name="wq", tag="w")
        nc.sync.dma_start(wq, wqp[:, :, :])
        wke = wpool.tile([P, CS, D], BF16, name="wke", tag="w")
        nc.sync.dma_start(wke, wkep[:, :, :])

        ident32 = sm[:, SM_ID32:SM_ID32 + 128]
        ident16 = smb[:, SB_ID16:SB_ID16 + 128]
        onesb = smb[0:1, SB_ONES:SB_ONES + 128]
        rwdT = smb[:, SB_RWD:SB_RWD + 400].rearrange("p (s t) -> p s t", s=DS)
        wb0_sb = smb[:, SB_WB0:SB_WB0 + 32].rearrange("p (s h) -> p s h", s=DS)
        fill = smb[:, SB_FILL:SB_FILL + WROW]
        eps_sb = persist.tile([P, 1], F32)
        nc.vector.memset(eps_sb, EPS)

        # scratch fill+band SBUF image: [P, NK, WROW] bf16 (23KB/partition)
        fb = persist.tile([P, NK, WROW], BF16)
        for k in range(NK):
            nc.vector.tensor_copy(fb[:, k, 0:127], fill[:, 0:127])
            nc.vector.tensor_copy(fb[:, k, WIN:WROW], fill[:, WIN:WROW])

        # --------------------------- LN helper --------------------------------
        def layernorm_to_T(x_tiles, gcol, bcol, lnT_out, name):
            for tt in range(TS):
                xt = x_tiles[:, tt, :]
                stats = work.tile([P, 6], F32, name=f"{name}st{tt}", tag="lnst")
                nc.vector.bn_stats(out=stats, in_=xt)
                mv = work.tile([P, 2], F32, name=f"{name}mv{tt}", tag="lnmv")
                nc.vector.bn_aggr(out=mv, in_=stats)
                rstd = work.tile([P, 1], F32, name=f"{name}rs{tt}", tag="lnrs")
                nc.scalar.activation(out=rstd, in_=mv[:, 1:2], func=AF.Sqrt,
                                     bias=eps_sb, scale=1.0)
                nc.vector.reciprocal(rstd, rstd)
                xn = work.tile([P, D], F32, name=f"{name}xn{tt}", tag="lnxn")
                nc.vector.tensor_scalar(xn, xt, mv[:, 0:1], rstd,
                                        op0=ALU.subtract, op1=ALU.mult)
                for es in range(DS):
                    tp = psum.tile([P, P], F32, name=f"{name}tp", tag="pp")
                    nc.tensor.transpose(tp, xn[:, es * P:(es + 1) * P], ident32)
                    nc.vector.tensor_scalar(
                        lnT_out[:, es, tt * P:(tt + 1) * P], tp,
                        gcol[:, es:es + 1], bcol[:, es:es + 1],
                        op0=ALU.mult, op1=ALU.add)

        # ------------------------- LN0 + block0 ------------------------------
        ln0T = persist.tile([P, DS, T], BF16, name="ln0T", tag="lnT")
        layernorm_to_T(vals, sm[:, SM_LN0G:SM_LN0G + DS],
                       sm[:, SM_LN0B:SM_LN0B + DS], ln0T, "ln0")

        xT = persist.tile([P, CS, T], BF16, name="xT", tag="xT")
        for cs_ in range(CS):
            pp = psacc.tile([P, T], F32, name="h0pp", tag="acc")
            for es in range(DS):
                nc.tensor.matmul(pp, wh0[:, es, cs_ * P:(cs_ + 1) * P],
                                 ln0T[:, es, :],
                                 start=(es == 0), stop=(es == DS - 1))
            nc.scalar.activation(out=xT[:, cs_, :], in_=pp, func=AF.Relu,
                                 bias=sm[:, SM_BH0 + cs_:SM_BH0 + cs_ + 1],
                                 scale=1.0)

        # --------------------------- projections -----------------------------
        def project_T(w_sb, dest, boff, scale):
            """dest [128(d), DS, T] (bf16) = scale*((x @ w).T + b)."""
            accs = [psacc.tile([P, T], F32, name=f"pa{d}", tag="acc")
                    for d in range(DS)]
            for cs_ in range(CS):
                for dsub in range(DS):
                    nc.tensor.matmul(
                        accs[dsub], w_sb[:, cs_, dsub * P:(dsub + 1) * P],
                        xT[:, cs_, :],
                        start=(cs_ == 0), stop=(cs_ == CS - 1))
            for dsub in range(DS):
                nc.scalar.activation(out=dest[:, dsub, :], in_=accs[dsub],
                                     func=AF.Identity,
                                     bias=sm[:, boff + dsub:boff + dsub + 1],
                                     scale=scale)

        qT = persist.tile([P, DS, T], BF16)      # holds q/8 transposed
        project_T(wq, qT, SM_BQ, 0.125)          # bias pre-scaled on host

        # ------------------- Db tiles -> scratch write ------------------------
        # fb[:, k, 127:227] = Db for k = ti*H + h
        hd = lambda h: (h % 2) * DIM
        for ti in range(TS):
            for h in range(H):
                dbp = psum.tile([P, BW], F32, name="dbp", tag="pp")
                nc.tensor.matmul(
                    dbp, qT[hd(h):hd(h) + DIM, h // 2, ti * P:(ti + 1) * P],
                    rwdT[hd(h):hd(h) + DIM, h // 2, :],
                    start=True, stop=False)
                nc.tensor.matmul(
                    dbp, onesb,
                    smb[0:1, SB_E1D + h * BW:SB_E1D + (h + 1) * BW],
                    start=False, stop=True)
                nc.vector.tensor_copy(fb[:, ti * H + h, 127:227], dbp)

        scr = dram.tile([P * ROWS], BF16, name="scr")
        nc.scalar.dma_start(
            bass.AP(tensor=scr.tensor, offset=scr.offset,
                    ap=[[ROWS, P], [WROW, NK], [1, WROW]]),
            fb)
        # skewed window read: win[p, k, w] = scr[p*ROWS + k*WROW + 127 + w - p]
        wins = []
        for ti in range(TS):
            wr = work.tile([P, H, WIN], BF16, name=f"win{ti}", tag=f"win{ti}",
                           bufs=1)
            nc.scalar.dma_start(
                wr,
                bass.AP(tensor=scr.tensor,
                        offset=scr.offset + ti * H * WROW + 127,
                        ap=[[ROWS - 1, P], [WROW, H], [1, WIN]]))
            wins.append(wr)

        # ------------------- keT, kv, bias0 (+mask) ---------------------------
        keT = persist.tile([P, DS, T], BF16)
        project_T(wke, keT, SM_BKE, 1.0)

        wkv = wpool.tile([P, CS, D], BF16, name="wkv", tag="w")
        nc.sync.dma_start(wkv, wkvp[:, :, :])
        kv = persist.tile([P, TS, D], BF16)
        kvaccs = [psacc.tile([P, D], F32, name=f"kva{t}", tag="acc")
                  for t in range(TS)]
        for cs_ in range(CS):
            for tt in range(TS):
                nc.tensor.matmul(kvaccs[tt],
                                 xT[:, cs_, tt * P:(tt + 1) * P],
                                 wkv[:, cs_, :],
                                 start=(cs_ == 0), stop=(cs_ == CS - 1))
        for tt in range(TS):
            nc.vector.tensor_copy(kv[:, tt, :], kvaccs[tt])  # bkv via v1 below

        # bias0 rows [H, T] + maskbias folded in; roundtrip to [P, H, T] bcast
        b0p = psum.tile([H, T], F32, name="b0p", tag="pp")
        for c in range(DS):
            nc.tensor.matmul(b0p, wb0_sb[:, c, :], keT[:, c, :],
                             start=(c == 0), stop=False)
        nc.tensor.matmul(b0p, onesb[:, 0:H], smb[0:1, SB_MASK:SB_MASK + T],
                         start=False, stop=True)
        b0m = work.tile([H, T], BF16, name="b0m", tag="b0m", bufs=1)
        nc.vector.tensor_copy(b0m, b0p)
        b0d = dram.tile([H * T], BF16, name="b0d")
        nc.scalar.dma_start(
            bass.AP(tensor=b0d.tensor, offset=b0d.offset, ap=[[T, H], [1, T]]),
            b0m)
        b0bc = persist.tile([P, H, T], BF16)
        nc.scalar.dma_start(
            b0bc,
            bass.AP(tensor=b0d.tensor, offset=b0d.offset,
                    ap=[[0, P], [T, H], [1, T]]))

        # ------------------------------ attention -----------------------------
        # Per ti, three phases so the PE queue never blocks on DVE/ACT:
        # (1) all 8 heads' score matmuls (+DVE bias adds, ACT exp),
        # (2) all probability-tile transposes (DVE copies trail),
        # (3) all attn@v accumulation matmuls (+ACT rz scale-out).
        attn_out = persist.tile([P, TS, D], F32)
        for ti in range(TS):
            nj = (ti + 1) * P
            j0 = ti * P - 99
            ats, rzs = [], []
            for h in range(H):
                sp = psacc.tile([P, T], F32, name="sp", tag="acc")
                nc.tensor.matmul(
                    sp[:, 0:nj],
                    qT[hd(h):hd(h) + DIM, h // 2, ti * P:(ti + 1) * P],
                    keT[hd(h):hd(h) + DIM, h // 2, 0:nj],
                    start=True, stop=True)
                if ti == 0:
                    nc.vector.tensor_tensor(sp[:, 0:P], sp[:, 0:P],
                                            wins[0][:, h, 99:WIN], ALU.add)
                else:
                    nc.vector.tensor_tensor(sp[:, j0:j0 + WIN],
                                            sp[:, j0:j0 + WIN],
                                            wins[ti][:, h, :], ALU.add)
                nc.vector.tensor_tensor(sp[:, 0:nj], sp[:, 0:nj],
                                        b0bc[:, h, 0:nj], ALU.add)
                at = work.tile([P, T], BF16, name=f"at{h}", tag=f"at{h}",
                               bufs=1)
                zs = work.tile([P, 1], F32, name=f"zs{h}", tag=f"zs{h}",
                               bufs=1)
                nc.scalar.activation(out=at[:, 0:nj], in_=sp[:, 0:nj],
                                     func=AF.Exp, bias=0.0, scale=1.0,
                                     accum_out=zs)
                rz = work.tile([P, 1], F32, name=f"rz{h}", tag=f"rz{h}",
                               bufs=1)
                nc.vector.reciprocal(rz, zs)
                ats.append(at)
                rzs.append(rz)
            atTs = work.tile([P, H, (ti + 1) * P], BF16, name="atTs",
                             tag="atTs", bufs=1)
            for h in range(H):
                for js in range(ti + 1):
                    tp = psum.tile([P, P], BF16, name="attp", tag="pp")
                    nc.tensor.transpose(tp, ats[h][:, js * P:(js + 1) * P],
                                        ident16)
                    nc.vector.tensor_copy(atTs[:, h, js * P:(js + 1) * P], tp)
            for h in range(H):
                op = pso.tile([P, DIM], F32, name="avp", tag="avp")
                for js in range(ti + 1):
                    nc.tensor.matmul(op, atTs[:, h, js * P:(js + 1) * P],
                                     kv[:, js, h * DIM:(h + 1) * DIM],
                                     start=(js == 0), stop=(js == ti))
                nc.scalar.activation(
                    out=attn_out[:, ti, h * DIM:(h + 1) * DIM], in_=op,
                    func=AF.Identity, bias=0.0, scale=rzs[h])

        # ------------------------ residual + block1 ---------------------------
        wh1 = wpool.tile([P, DS, HID], BF16, name="wh1", tag="w")
        nc.sync.dma_start(wh1, wh1p[:, :, :])
        v1 = persist.tile([P, TS, D], F32)
        for tt in range(TS):
            nc.vector.tensor_add(v1[:, tt, :], vals[:, tt, :],
                                 attn_out[:, tt, :])
            nc.vector.tensor_add(v1[:, tt, :], v1[:, tt, :],
                                 sm[:, SM_BKV:SM_BKV + D])
        ln1T = persist.tile([P, DS, T], BF16, name="ln1T", tag="lnT")
        layernorm_to_T(v1, sm[:, SM_LN1G:SM_LN1G + DS],
                       sm[:, SM_LN1B:SM_LN1B + DS], ln1T, "ln1")

        x1T = persist.tile([P, CS, T], BF16, name="x1T", tag="xT")
        for cs_ in range(CS):
            pp = psacc.tile([P, T], F32, name="h1pp", tag="acc")
            for es in range(DS):
                nc.tensor.matmul(pp, wh1[:, es, cs_ * P:(cs_ + 1) * P],
                                 ln1T[:, es, :],
                                 start=(es == 0), stop=(es == DS - 1))
            nc.scalar.activation(out=x1T[:, cs_, :], in_=pp, func=AF.Relu,
                                 bias=sm[:, SM_BH1 + cs_:SM_BH1 + cs_ + 1],
                                 scale=1.0)

        wo1 = wpool.tile([P, CS, D], BF16, name="wo1", tag="w")
        nc.sync.dma_start(wo1, wo1p[:, :, :])
        o1accs = [psacc.tile([P, D], F32, name=f"o1a{t}", tag="acc")
                  for t in range(TS)]
        for cs_ in range(CS):
            for tt in range(TS):
                nc.tensor.matmul(o1accs[tt],
                                 x1T[:, cs_, tt * P:(tt + 1) * P],
                                 wo1[:, cs_, :],
                                 start=(cs_ == 0), stop=(cs_ == CS - 1))
        for tt in range(TS):
            fin = work.tile([P, D], F32, name="fin", tag="fin")
            nc.vector.tensor_add(fin, o1accs[tt], v1[:, tt, :])
            nc.vector.tensor_add(fin, fin, sm[:, SM_BO1:SM_BO1 + D])
            nc.sync.dma_start(out[:, tt, :], fin)

    if not nc.is_finalized():
        nc.finalize()
    return nc


def _pcol(v):
    """[D] -> [128, D//128] partition-major columns."""
    return np.ascontiguousarray(v.reshape(-1, P).T)


def _pmajor(w, rows_per_part):
    """[(s p), c] -> [128, s, c]."""
    s = rows_per_part
    return np.ascontiguousarray(
        w.reshape(s, P, w.shape[1]).transpose(1, 0, 2))


def build_in_maps(inputs):
    f32 = lambda x: np.asarray(x, dtype=np.float32)
    bf = lambda x: np.ascontiguousarray(x).astype(NPBF)

    rel101 = f32(inputs["rel_enc"])[:L + 1]                     # [101, D]
    wkr = f32(inputs["wkr"])
    wb1 = f32(inputs["wb1"])
    RW = (rel101 @ wkr).T                                       # [D, 101]
    rwd8 = 8.0 * (RW[:, 1:] - RW[:, 0:1])                       # [D, 100]
    rwdT = rwd8.reshape(DS, P, BW).transpose(1, 0, 2).reshape(P, DS * BW)
    E1 = rel101 @ wkr @ wb1                                     # [101, H]
    e1d = (E1[1:] - E1[0:1]).T                                  # [H, 100]

    smalls = np.zeros((P, NS), np.float32)
    smalls[:, SM_LN0G:SM_LN0G + DS] = _pcol(f32(inputs["ln0_g"]))
    smalls[:, SM_LN0B:SM_LN0B + DS] = _pcol(f32(inputs["ln0_b"]))
    smalls[:, SM_LN1G:SM_LN1G + DS] = _pcol(f32(inputs["ln1_g"]))
    smalls[:, SM_LN1B:SM_LN1B + DS] = _pcol(f32(inputs["ln1_b"]))
    smalls[:, SM_BH0:SM_BH0 + CS] = _pcol(f32(inputs["b_h0"]))
    smalls[:, SM_BH1:SM_BH1 + CS] = _pcol(f32(inputs["b_h1"]))
    smalls[:, SM_BQ:SM_BQ + DS] = _pcol(f32(inputs["bq"])) * 0.125
    smalls[:, SM_BKE:SM_BKE + DS] = _pcol(f32(inputs["bke"]))
    smalls[:, SM_ID32:SM_ID32 + P] = np.eye(P, dtype=np.float32)
    smalls[:, SM_BKV:SM_BKV + D] = np.tile(f32(inputs["bkv"]), (P, 1))
    smalls[:, SM_BO1:SM_BO1 + D] = np.tile(f32(inputs["b_o1"]), (P, 1))

    mask = np.asarray(inputs["values_mask"])
    maskbias = np.where(mask, 0.0, NEG).astype(np.float32)      # [B, T]

    smb_base = np.zeros((P, NSB), np.float32)
    smb_base[:, SB_ID16:SB_ID16 + P] = np.eye(P, dtype=np.float32)
    smb_base[0, SB_ONES:SB_ONES + P] = 1.0
    smb_base[:, SB_RWD:SB_RWD + DS * BW] = rwdT
    smb_base[0, SB_E1D:SB_E1D + H * BW] = e1d.reshape(-1)
    smb_base[:, SB_WB0:SB_WB0 + DS * H] = _pmajor(f32(inputs["wb0"]), DS
                                                  ).reshape(P, DS * H)
    fill = np.zeros(WROW, np.float32)
    fill[WIN:] = NEG
    smb_base[:, SB_FILL:SB_FILL + WROW] = fill[None, :]

    shared = {
        "smalls": smalls,
        "wh0p": bf(_pmajor(f32(inputs["w_h0"]), DS)),
        "wqp": bf(_pmajor(f32(inputs["wq"]), CS)),
        "wkep": bf(_pmajor(f32(inputs["wke"]), CS)),
        "wkvp": bf(_pmajor(f32(inputs["wkv"]), CS)),
        "wh1p": bf(_pmajor(f32(inputs["w_h1"]), DS)),
        "wo1p": bf(_pmajor(f32(inputs["w_o1"]), CS)),
    }

    vals = f32(inputs["values"])
    in_maps = []
    for b in range(B):
        m = dict(shared)
        m["values_b"] = np.ascontiguousarray(
            vals[b].reshape(TS, P, D).transpose(1, 0, 2))
        smb = smb_base.copy()
        smb[0, SB_MASK:SB_MASK + T] = maskbias[b]
        m["smallsb_b"] = bf(smb)
        in_maps.append(m)
    return in_maps


_NC_CACHE = None


def kernel(**inputs) -> np.ndarray:
    global _NC_CACHE
    if _NC_CACHE is None:
        _NC_CACHE = build_nc()
    nc = _NC_CACHE

    from concourse.bass_utils import run_bass_kernel_spmd

    in_maps = build_in_maps(inputs)
    res = run_bass_kernel_spmd(nc, in_maps, core_ids=list(range(B)))
    return np.stack(
        [res.results[b]["out_b"].transpose(1, 0, 2).reshape(T, D)
         for b in range(B)], axis=0)


if __name__ == "__main__":
    nc = build_nc()
    print("built ok")
